# revision 2
# baseline (speedup 1.0000x reference)
"""Trainium2 Bass kernel for the DifferentiableTree module.

Math (per sample s, heap-ordered perfect binary tree, depth 10):
  gate[s, i] = 0.5*(1 + erf((thr[i] - mean[s, f(i)]) / (std[s, f(i)]*sqrt(2))))
  w = path products over levels; pred = (w @ leaf_values) row-normalized.

Kernel strategy (data-parallel over samples, 8 cores x 8192 samples):
  - Host precomputes a selection/threshold matrix R [128, 1024] so that one
    PE matmul per 128-sample chunk produces every erf argument:
      arg[s, col] = thr[h]*a[s, f(h)] - b[s, f(h)],
    with a = 1/(sqrt(2)*std), b = mean*a, lhsT = [a^T; b^T] from a PE
    transpose of the X chunk.  Column layout: level l >= 1 of the tree at
    cols [2^l, 2^(l+1)), root at col 1, col 0 dummy -- this keeps every
    tree-level slice 4-byte aligned for the DVE 2x fp16 mode.
  - ScalarE evaluates erf -> E (fp16).  The 0.5*(1+e) gate affine is folded
    away by computing unnormalized weights prod(1 +/- e) = 2^10 * w (the
    final row normalization cancels any global scale).
  - Tree combine on VectorE via scalar_tensor_tensor, 16 chunks packed per
    instruction: hi = (E+1)*lo ; lo = 2*lo - hi.
  - W is transposed leaf-major via DMA xbar transposes (fp16), then PE
    contracts with the (host-permuted) leaf matrix, with an extra 129th
    column holding leaf row-sums so the normalizer comes out of the same
    matmul.  ScalarE applies 1/rowsum via exp/ln-free rsqrt+square.
"""

import numpy as np

N = 65536
N_CORES = 8
NS = N // N_CORES          # 8192 samples per core
F = 64
DEPTH = 10
NL = 2 ** DEPTH            # 1024 leaves
NCLS = 128
CHUNKS = NS // 128         # 64
PACK = 16                  # chunks per supertile
SUPER = CHUNKS // PACK     # 4
SQRT2 = 1.4142135623730951

_CACHE = {}


def _build_tables(features, thresholds, leaf_values):
    """R [128,1024] f32, LE [128, 8, 129] f16 (chunk-major leaf matrix)."""
    features = np.asarray(features).astype(np.int64)
    thresholds = np.asarray(thresholds, dtype=np.float64)
    leaf_values = np.asarray(leaf_values, dtype=np.float64)

    # heap node reached at level l by path bits b_i = bit i of j
    # (b=1 -> g branch = left child 2h+1; b=0 -> right child 2h+2)
    R = np.zeros((128, NL), dtype=np.float32)
    for l in range(DEPTH):
        nj = 2 ** l
        j = np.arange(nj)
        h = np.zeros(nj, dtype=np.int64)
        for i in range(l):
            b = (j >> i) & 1
            h = np.where(b == 1, 2 * h + 1, 2 * h + 2)
        cols = np.full(1, 1) if l == 0 else 2 ** l + j
        f = features[h]
        R[f, cols] = thresholds[h].astype(np.float32)
        R[64 + f, cols] = -1.0

    # kernel leaf j -> reference leaf: digit s_l = 1 - bit_l(j), MSB-first
    j = np.arange(NL)
    Lref = np.zeros(NL, dtype=np.int64)
    for l in range(DEPTH):
        Lref += (1 - ((j >> l) & 1)) * 2 ** (DEPTH - 1 - l)
    Lperm = leaf_values[Lref]                        # [1024, 128]
    Lext = np.concatenate([Lperm, Lperm.sum(axis=1, keepdims=True)], axis=1)
    LE = Lext.reshape(8, 128, 129).transpose(1, 0, 2)  # [128, 8, 129]
    return R, LE.astype(np.float16)


def _build_nc():
    import concourse.bacc as bacc
    import concourse.tile as tile
    from concourse import mybir

    f32 = mybir.dt.float32
    f32r = mybir.dt.float32r
    f16 = mybir.dt.float16
    AF = mybir.ActivationFunctionType
    OP = mybir.AluOpType

    nc = bacc.Bacc("TRN2", target_bir_lowering=False, debug=False,
                   num_devices=N_CORES)
    X_d = nc.dram_tensor("X", [NS, 128], f32, kind="ExternalInput")
    R_d = nc.dram_tensor("R", [128, NL], f32r, kind="ExternalInput")
    LE_d = nc.dram_tensor("LE", [128, 8, 129], f16, kind="ExternalInput")
    ID_d = nc.dram_tensor("ID", [128, 128], f32, kind="ExternalInput")
    O_d = nc.dram_tensor("OUT", [NS, 128], f32, kind="ExternalOutput")

    with tile.TileContext(nc) as tc:
        with (
            tc.tile_pool(name="consts", bufs=1) as consts,
            tc.tile_pool(name="xp", bufs=4) as xp,
            tc.tile_pool(name="abp", bufs=2) as abp,
            tc.tile_pool(name="r1p", bufs=2) as r1p,
            tc.tile_pool(name="ep", bufs=2) as ep,
            tc.tile_pool(name="wp", bufs=2) as wp,
            tc.tile_pool(name="wtp", bufs=1) as wtp,
            tc.tile_pool(name="op", bufs=4) as op,
            tc.tile_pool(name="nrm", bufs=8) as nrm,
            tc.tile_pool(name="xtps", bufs=2, space="PSUM") as xtps,
            tc.tile_pool(name="argps", bufs=2, space="PSUM") as argps,
            tc.tile_pool(name="predps", bufs=2, space="PSUM") as predps,
        ):
            r_sb = consts.tile([128, NL], f32r)
            nc.sync.dma_start(out=r_sb, in_=R_d[:, :])
            le_sb = consts.tile([128, 8, 129], f16)
            nc.sync.dma_start(out=le_sb, in_=LE_d[:, :, :])
            id_sb = consts.tile([128, 128], f32)
            nc.sync.dma_start(out=id_sb, in_=ID_d[:, :])

            for s in range(SUPER):
                E = ep.tile([128, PACK, NL], f16)
                W = wp.tile([128, PACK, NL], f16)
                wT = wtp.tile([128, PACK, 8, 128], f16)

                for q in range(PACK // 4):
                    xt = xtps.tile([128, 4, 128], f32)
                    ab = abp.tile([128, 4, 128], f32r)
                    r1 = r1p.tile([64, 4, 128], f32)
                    for k in range(4):
                        g = s * PACK + q * 4 + k
                        x = xp.tile([128, 128], f32)
                        nc.sync.dma_start(
                            out=x, in_=X_d[g * 128:(g + 1) * 128, :])
                        nc.tensor.transpose(xt[:, k, :], x, id_sb)
                    # a = 1/(sqrt(2)*std) via rsqrt+square; b = mean*a
                    nc.scalar.activation(r1, xt[64:128, :, :],
                                         AF.Abs_reciprocal_sqrt)
                    nc.scalar.activation(ab[0:64, :, :], r1, AF.Square,
                                         scale=2.0 ** -0.25)
                    nc.vector.tensor_mul(ab[64:128, :, :], xt[0:64, :, :],
                                         ab[0:64, :, :])
                    for k in range(4):
                        c = q * 4 + k
                        arg = argps.tile([128, NL], f32)
                        abr = ab[:, k, :]
                        nc.tensor.matmul(arg[:, 0:512], abr, r_sb[:, 0:512])
                        nc.tensor.matmul(arg[:, 512:1024], abr,
                                         r_sb[:, 512:1024])
                        nc.scalar.activation(E[:, c, :], arg, AF.Erf)

                # tree combine: unnormalized w, prod(1 +/- e)
                nc.vector.tensor_scalar(W[:, :, 0:1], E[:, :, 1:2],
                                        -1.0, 1.0, OP.mult, OP.add)
                nc.vector.tensor_scalar(W[:, :, 1:2], E[:, :, 1:2],
                                        1.0, 1.0, OP.mult, OP.add)
                for l in range(1, DEPTH):
                    d = 2 ** l
                    lo = W[:, :, 0:d]
                    hi = W[:, :, d:2 * d]
                    e = E[:, :, d:2 * d]
                    nc.vector.scalar_tensor_tensor(
                        hi, e, 1.0, lo, OP.add, OP.mult)
                    nc.vector.scalar_tensor_tensor(
                        lo, lo, 2.0, hi, OP.mult, OP.subtract)

                for c in range(PACK):
                    g = s * PACK + c
                    for t in range(8):
                        nc.sync.dma_start_transpose(
                            wT[:, c, t, :], W[:, c, t * 128:(t + 1) * 128])
                    pred = predps.tile([128, 129], f32)
                    for t in range(8):
                        nc.tensor.matmul(pred, wT[:, c, t, :], le_sb[:, t, :],
                                         start=(t == 0), stop=(t == 7))
                    rs = nrm.tile([128, 1], f32)
                    rcp = nrm.tile([128, 1], f32)
                    nc.scalar.activation(rs, pred[:, 128:129],
                                         AF.Abs_reciprocal_sqrt)
                    nc.scalar.activation(rcp, rs, AF.Square)
                    o = op.tile([128, 128], f32)
                    nc.scalar.activation(o, pred[:, 0:128], AF.Copy,
                                         scale=rcp)
                    nc.sync.dma_start(
                        out=O_d[g * 128:(g + 1) * 128, :], in_=o)

    nc.compile()
    return nc


def kernel(X, features, thresholds, leaf_values, trace=False):
    from concourse.bass_utils import run_bass_kernel_spmd

    X = np.ascontiguousarray(np.asarray(X, dtype=np.float32))
    R, LE = _build_tables(features, thresholds, leaf_values)
    ID = np.eye(128, dtype=np.float32)

    if "nc" not in _CACHE:
        _CACHE["nc"] = _build_nc()
    nc = _CACHE["nc"]

    in_maps = [
        {"X": X[c * NS:(c + 1) * NS], "R": R, "LE": LE, "ID": ID}
        for c in range(N_CORES)
    ]
    res = run_bass_kernel_spmd(nc, in_maps, core_ids=list(range(N_CORES)),
                               trace=trace)
    out = np.concatenate([res.results[c]["OUT"] for c in range(N_CORES)],
                         axis=0)
    _CACHE["last_results"] = res
    return out
